# revision 1
# baseline (speedup 1.0000x reference)
"""ConvSpikingBlock Trainium2 kernel (8 NeuronCores, data-parallel over batch).

Algorithm (per core, 2 of 16 batches):
  phase 1 NEFF: 3x3 conv (as one K=36 matmul per frame-half via a shifted-row
    rhs layout) -> bn_stats per frame -> raw stats out; host combines stats
    across cores/partitions in fp64 and folds BN affine into the conv weights.
  phase 2 NEFF: conv with folded weights accumulates directly onto the PSUM
    resident membrane state: for each step
      ACT:  bank = beta * v_prev + bias''           (PSUM->PSUM, per-part bias)
      PE :  bank += W'_hi.T @ rhs_hi  (f32r, exact: operands 12-bit mantissas)
      PE :  bank += W'corr.T @ rhs_corr (bf16 correction -> full fp32 accuracy)
      DVE:  s = (bank > theta) -> spikes (f32r {0,1}) -> DMA to output
      PE :  bank += (-theta I) @ s                  (reset; v stays in PSUM)
  Spikes are DMA-scattered straight into the final (B,T,C,H,W) layout.

Precision: conv inputs/weights are split hi(12-bit mantissa, f32r full-rate
matmul is exact on them) + lo(bf16 correction terms), recovering ~fp32 conv.
"""

import os
import sys

sys.path.insert(0, "/opt/trn_rl_repo")

import ml_dtypes
import numpy as np

import bass_rust
import concourse.bacc as bacc
import concourse.tile as tile
from concourse import mybir
from concourse.bass_utils import run_bass_kernel_spmd

F32 = mybir.dt.float32
F32R = mybir.dt.float32r
BF16 = mybir.dt.bfloat16
BF = ml_dtypes.bfloat16

B, T, CIN, H, W = 16, 20, 2, 64, 64
COUT, KS = 32, 3
NC_ = 8
BLOC = B // NC_          # 2 batches per core
NF = BLOC * T            # 40 frames per core
EPS = 1e-5
KH = 36                  # hi-set contraction rows (6 row6 x 3 kw x 2 cin)
KC = 72                  # corr-set rows (lo ; full)
NPIX = 1024              # free size per frame (16 groups x 64 cols)

LAST_EXEC_NS = {}


def _trunc12(a):
    return (np.ascontiguousarray(a).view(np.uint32) & np.uint32(0xFFFFF000)).view(
        np.float32
    )


def _ap(base, dims, extra=0):
    ap = base.copy()
    ap.ap = bass_rust.VecI64Pair(dims)
    ap.offset = base.offset + extra
    return ap


SIM_INIT = bool(os.environ.get("SIM_INIT"))


def _build_rhs_dmas(nc, dst_slot36, src_frame_ap, elem_rowsz):
    """Emit 2 DMAs (one per cin) filling a 36-row rhs slot from a padded
    (2,66,66) source frame AP. dst_slot36 = AP of rows [k0, k0+36) of an SBUF
    tile; elem_rowsz = dst tile row size in elements (partition step)."""
    for cin in range(2):
        for kw in range(3):
            out_ap = _ap(
                dst_slot36,
                [[6 * elem_rowsz, 6], [64, 16], [1, 64]],
                extra=(2 * kw + cin) * elem_rowsz,
            )
            in_ap = _ap(
                src_frame_ap,
                [[66, 6], [264, 16], [1, 64]],
                extra=cin * 66 * 66 + kw,
            )
            nc.sync.dma_start(out_ap, in_ap)


def _w_block(w):
    """[36,128] weight block: k=(row6*6+kw*2+cin), m=(r_out*32+cout)."""
    wb = np.zeros((KH, 128), np.float64)
    for r in range(4):
        for kh in range(KS):
            k6 = r + kh
            for kw in range(KS):
                for cin in range(CIN):
                    wb[k6 * 6 + kw * 2 + cin, r::4] = w[:, cin, kh, kw]
    return wb


def _phase1(x_hi, x_lo_bf, x_fl_bf, wb):
    nc = bacc.Bacc("TRN2", target_bir_lowering=False, debug=False, num_devices=NC_)
    xh_d = nc.dram_tensor("x_hi", [BLOC, T, CIN, 66, 66], F32R, kind="ExternalInput")
    xl_d = nc.dram_tensor("x_lo", [BLOC, T, CIN, 66, 66], BF16, kind="ExternalInput")
    xf_d = nc.dram_tensor("x_fl", [BLOC, T, CIN, 66, 66], BF16, kind="ExternalInput")
    wh_d = nc.dram_tensor("w_hi", [100, 128], F32R, kind="ExternalInput")
    wc_d = nc.dram_tensor("w_c", [KC, 128], BF16, kind="ExternalInput")
    st_d = nc.dram_tensor("stats", [128, NF * 12], F32, kind="ExternalOutput")

    with tile.TileContext(nc) as tc:
        with (
            tc.tile_pool(name="res", bufs=1) as res,
            tc.tile_pool(name="corrp", bufs=6) as corrp,
            tc.tile_pool(name="psum", bufs=4, space="PSUM") as psum,
        ):
            wh = res.tile([100, 128], F32R)
            nc.sync.dma_start(wh[:], wh_d[:])
            wc = res.tile([KC, 128], BF16)
            nc.sync.dma_start(wc[:], wc_d[:])
            statsbuf = res.tile([128, NF * 12], F32)

            hi_tiles = [res.tile([100, NPIX], F32R, name=f"hi{j}") for j in range((NF + 1) // 2)]
            if SIM_INIT:
                for ht in hi_tiles:
                    nc.gpsimd.memset(ht[:].bitcast(F32), 0.0)

            for f in range(NF):
                b, t = divmod(f, T)
                k0 = 64 * (f % 2)
                slot = hi_tiles[f // 2][k0 : k0 + KH, :]
                _build_rhs_dmas(nc, slot, xh_d[b, t].flatten(), NPIX)

                corr = corrp.tile([KC, NPIX], BF16)
                if SIM_INIT:
                    nc.gpsimd.memset(corr[:].bitcast(mybir.dt.uint16), 0)
                _build_rhs_dmas(nc, corr[0:KH, :], xl_d[b, t].flatten(), NPIX)
                _build_rhs_dmas(nc, corr[KH:KC, :], xf_d[b, t].flatten(), NPIX)

                acc = psum.tile([128, NPIX], F32)
                for hf in range(2):
                    cols = slice(hf * 512, hf * 512 + 512)
                    nc.tensor.matmul(
                        acc[:, cols], wh[k0 : k0 + KH, :], slot[:, cols],
                        start=True, stop=False,
                    )
                    nc.tensor.matmul(
                        acc[:, cols], wc[:], corr[:, cols],
                        start=False, stop=True, skip_group_check=True,
                    )
                for hf in range(2):
                    nc.vector.bn_stats(
                        statsbuf[:, f * 12 + hf * 6 : f * 12 + hf * 6 + 6],
                        acc[:, hf * 512 : hf * 512 + 512],
                    )
            nc.sync.dma_start(st_d[:], statsbuf[:])
    nc.compile()
    return nc


def _phase2(negI_lo_needed):
    nc = bacc.Bacc("TRN2", target_bir_lowering=False, debug=False, num_devices=NC_)
    xh_d = nc.dram_tensor("x_hi", [BLOC, T, CIN, 66, 66], F32R, kind="ExternalInput")
    xl_d = nc.dram_tensor("x_lo", [BLOC, T, CIN, 66, 66], BF16, kind="ExternalInput")
    xf_d = nc.dram_tensor("x_fl", [BLOC, T, CIN, 66, 66], BF16, kind="ExternalInput")
    wh_d = nc.dram_tensor("w_hi", [100, 128], F32R, kind="ExternalInput")
    wc_d = nc.dram_tensor("w_c", [KC, 128], BF16, kind="ExternalInput")
    ni_d = nc.dram_tensor("negI", [128, 128], F32R, kind="ExternalInput")
    bi_d = nc.dram_tensor("bias", [128, 1], F32, kind="ExternalInput")
    vi_d = nc.dram_tensor("vinit", [BLOC, 128, NPIX], F32, kind="ExternalInput")
    s0_d = nc.dram_tensor("sinit", [BLOC, 128, NPIX], F32R, kind="ExternalInput")
    out_d = nc.dram_tensor("spk", [BLOC, T, COUT, H, W], F32, kind="ExternalOutput")

    BETA = _phase2.beta
    THETA = _phase2.theta

    with tile.TileContext(nc) as tc:
        with (
            tc.tile_pool(name="res", bufs=1) as res,
            tc.tile_pool(name="corrp", bufs=6) as corrp,
            tc.tile_pool(name="sp", bufs=8) as sp,
            tc.tile_pool(name="psum", bufs=1, space="PSUM") as psum,
        ):
            wh = res.tile([100, 128], F32R)
            nc.sync.dma_start(wh[:], wh_d[:])
            wc = res.tile([KC, 128], BF16)
            nc.sync.dma_start(wc[:], wc_d[:])
            negI = res.tile([128, 128], F32R)
            nc.sync.dma_start(negI[:], ni_d[:])
            bias = res.tile([128, 1], F32)
            nc.sync.dma_start(bias[:], bi_d[:])
            vinit = res.tile([128, BLOC * NPIX], F32)
            for b in range(BLOC):
                nc.sync.dma_start(vinit[:, b * NPIX : (b + 1) * NPIX], vi_d[b])
            sinit = res.tile([128, BLOC * NPIX], F32R)
            for b in range(BLOC):
                nc.sync.dma_start(sinit[:, b * NPIX : (b + 1) * NPIX], s0_d[b])

            hi_tiles = [res.tile([100, NPIX], F32R, name=f"hi{j}") for j in range((NF + 1) // 2)]
            if SIM_INIT:
                for ht in hi_tiles:
                    nc.gpsimd.memset(ht[:].bitcast(F32), 0.0)
            for f in range(NF):
                b, t = divmod(f, T)
                k0 = 64 * (f % 2)
                _build_rhs_dmas(nc, hi_tiles[f // 2][k0 : k0 + KH, :], xh_d[b, t].flatten(), NPIX)

            corr_tiles = {}
            for t in range(T):
                for b in range(BLOC):
                    f = b * T + t
                    corr = corrp.tile([KC, NPIX], BF16, name=f"corr{f}", tag="corr")
                    if SIM_INIT:
                        nc.gpsimd.memset(corr[:].bitcast(mybir.dt.uint16), 0)
                    _build_rhs_dmas(nc, corr[0:KH, :], xl_d[b, t].flatten(), NPIX)
                    _build_rhs_dmas(nc, corr[KH:KC, :], xf_d[b, t].flatten(), NPIX)
                    corr_tiles[f] = corr

            banks = [
                [
                    [psum.tile([128, 512], F32, name=f"bk{b}_{hf}_{g}") for g in range(2)]
                    for hf in range(2)
                ]
                for b in range(BLOC)
            ]
            zl = res.tile([1, 128], F32R)
            nc.vector.memset(zl[:].bitcast(F32), 0.0)
            zr = res.tile([1, 512], F32R)
            nc.vector.memset(zr[:].bitcast(F32), 0.0)
            for b in range(BLOC):
                for hf in range(2):
                    for g in range(2):
                        nc.tensor.matmul(
                            banks[b][hf][g][:], zl[:], zr[:], start=True, stop=True
                        )

            # out scatter strides (elements) in [BLOC,T,COUT,H,W]
            SB_, ST_, SC_ = T * COUT * H * W, COUT * H * W, H * W
            s_prev = {}
            for b in range(BLOC):
                for hf in range(2):
                    s_prev[(b, hf)] = sinit[:, b * NPIX + hf * 512 : b * NPIX + hf * 512 + 512]
            for t in range(T):
                for b in range(BLOC):
                    f = b * T + t
                    k0 = 64 * (f % 2)
                    for hf in range(2):
                        cur = banks[b][hf][t % 2]
                        cols = slice(hf * 512, hf * 512 + 512)
                        if t == 0:
                            vsrc = vinit[:, b * NPIX + hf * 512 : b * NPIX + hf * 512 + 512]
                        else:
                            vsrc = banks[b][hf][(t + 1) % 2][:]
                        nc.scalar.activation(
                            cur[:], vsrc,
                            mybir.ActivationFunctionType.Identity,
                            bias=bias[:], scale=BETA,
                        )
                        nc.tensor.matmul(
                            cur[:], negI[:], s_prev[(b, hf)],
                            start=False, stop=True, skip_group_check=True,
                        )
                        nc.tensor.matmul(
                            cur[:], wh[k0 : k0 + KH, :],
                            hi_tiles[f // 2][k0 : k0 + KH, cols],
                            start=False, stop=True, skip_group_check=True,
                        )
                        nc.tensor.matmul(
                            cur[:], wc[:], corr_tiles[f][:, cols],
                            start=False, stop=True, skip_group_check=True,
                        )
                        s = sp.tile([128, 512], F32R, name=f"s{f}_{hf}", tag="s")
                        nc.vector.tensor_scalar(
                            out=s[:], in0=cur[:], scalar1=THETA, scalar2=None,
                            op0=mybir.AluOpType.is_gt,
                        )
                        for r in range(4):
                            out_ap = _ap(
                                out_d.ap(),
                                [[SC_, 32], [256, 8], [1, 64]],
                                extra=b * SB_ + t * ST_ + hf * 2048 + r * 64,
                            )
                            in_ap = _ap(
                                s[:].bitcast(F32),
                                [[2048, 32], [64, 8], [1, 64]],
                                extra=r * 512,
                            )
                            nc.sync.dma_start(out_ap, in_ap)
                        s_prev[(b, hf)] = s[:]
    nc.compile()
    return nc


def kernel(x, mem_init, conv_w, conv_b, bn_gamma, bn_bias, beta, threshold):
    x = np.asarray(x, np.float32)
    mem_init = np.asarray(mem_init, np.float32)
    conv_w = np.asarray(conv_w, np.float32)
    bn_gamma = np.asarray(bn_gamma, np.float32)
    bn_bias = np.asarray(bn_bias, np.float32)
    betac = float(np.clip(np.float32(beta), 0.0, 1.0))
    theta = float(np.float32(threshold))

    # ---- host prep: padded hi/lo inputs
    xp = np.zeros((B, T, CIN, 66, 66), np.float32)
    xp[:, :, :, 1:65, 1:65] = x
    x_hi = _trunc12(xp)
    x_lo = (xp - x_hi).astype(BF)
    x_fl = xp.astype(BF)

    wb = _w_block(conv_w)  # [36,128] fp64

    def w_inputs(wb32):
        """hi (f32r, dup at 0/64) + corr (bf16 [72,128]) from fp32 block."""
        w_hi = _trunc12(wb32)
        w_lo = (wb32 - w_hi).astype(np.float32)
        whi_dup = np.zeros((100, 128), np.float32)
        whi_dup[0:KH] = w_hi
        whi_dup[64 : 64 + KH] = w_hi
        wc = np.zeros((KC, 128), BF)
        wc[0:KH] = w_hi.astype(BF)       # pairs with x_lo rows
        wc[KH:KC] = w_lo.astype(BF)      # pairs with x_fl rows
        return whi_dup, wc

    wh1, wc1 = w_inputs(wb.astype(np.float32))

    # ---- phase 1: stats
    nc1 = _phase1(x_hi, x_lo, x_fl, wb)
    in_maps1 = []
    for c in range(NC_):
        sl = slice(c * BLOC, (c + 1) * BLOC)
        in_maps1.append(
            {
                "x_hi": x_hi[sl], "x_lo": x_lo[sl], "x_fl": x_fl[sl],
                "w_hi": wh1, "w_c": wc1,
            }
        )
    import time as _time
    _t = _time.time()
    r1 = run_bass_kernel_spmd(nc1, in_maps1, core_ids=list(range(NC_)))
    LAST_EXEC_NS["phase1_wall"] = (_time.time() - _t) * 1e9

    # ---- host: combine stats (each 6-tuple: [cnt,mean,M2, cnt,mean,M2])
    tot_n = 0.0
    tot_s = np.zeros(COUT, np.float64)
    tot_q = np.zeros(COUT, np.float64)
    for c in range(NC_):
        st = r1.results[c]["stats"].astype(np.float64).reshape(128, NF * 2, 6)
        for half in (0, 3):
            cnt = st[:, :, half]
            mean = st[:, :, half + 1]
            m2 = st[:, :, half + 2]
            s = (cnt * mean).reshape(32, 4, -1).sum(axis=(1, 2))
            q = (m2 + cnt * mean * mean).reshape(32, 4, -1).sum(axis=(1, 2))
            tot_s += s
            tot_q += q
            tot_n += cnt.reshape(32, 4, -1).sum(axis=(1, 2))[0] / 1.0
    # tot_n accumulated per channel identically; recompute exactly:
    n_tot = float(B * T * H * W)
    mu = tot_s / n_tot
    var = tot_q / n_tot - mu * mu
    gp = bn_gamma.astype(np.float64) / np.sqrt(var + EPS)
    # reference normalizes y=conv+cb, but cb cancels: b'' = bn_bias - gp*mu
    bpp = bn_bias.astype(np.float64) - gp * mu
    wb2 = (wb * np.repeat(gp, 4)[None, :]).astype(np.float32)
    wh2, wc2 = w_inputs(wb2)

    bias128 = np.repeat(bpp, 4).astype(np.float32).reshape(128, 1)
    negI = _trunc12(-theta * np.eye(128, dtype=np.float32))

    def to_layout(a):
        # [B, C, H, W] -> [B, p=c*4+r, n=g*64+w] with h = 4g+r
        a = a.reshape(B, COUT, 16, 4, 64)
        return np.ascontiguousarray(a.transpose(0, 1, 3, 2, 4).reshape(B, 128, NPIX))

    v0 = to_layout(mem_init.astype(np.float32))
    s0 = to_layout((mem_init > theta).astype(np.float32))

    _phase2.beta = betac
    _phase2.theta = theta
    nc2 = _phase2(False)
    in_maps2 = []
    for c in range(NC_):
        sl = slice(c * BLOC, (c + 1) * BLOC)
        in_maps2.append(
            {
                "x_hi": x_hi[sl], "x_lo": x_lo[sl], "x_fl": x_fl[sl],
                "w_hi": wh2, "w_c": wc2, "negI": negI,
                "bias": bias128, "vinit": v0[sl], "sinit": s0[sl],
            }
        )
    _t = _time.time()
    r2 = run_bass_kernel_spmd(nc2, in_maps2, core_ids=list(range(NC_)))
    LAST_EXEC_NS["phase2_wall"] = (_time.time() - _t) * 1e9

    out = np.concatenate([r2.results[c]["spk"] for c in range(NC_)], axis=0)
    return out.astype(np.float32)



# revision 2
# speedup vs baseline: 47912.4893x; 47912.4893x over previous
"""ConvSpikingBlock Trainium2 kernel (8 NeuronCores, data-parallel over batch).

Per core (2 of 16 batches, 40 frames):
  phase 1 NEFF: conv via ONE fp16 matmul per frame-half (3-term split, see
    below) -> bn_stats per half -> raw stats out; host combines stats across
    cores in fp64 and folds the BN affine into the conv weights.
  phase 2 NEFF: conv with folded weights accumulates onto a PSUM-resident
    membrane; per step t, per batch b:
      ACT:  bank = beta * v_prev + b''         (per-partition bias, 1024 wide)
      PE :  bank += (-theta I) @ s_prev        (f32r reset, per 512 half)
      PE :  bank += W3.T @ rhs3[f]             (fp16 conv, per 512 half)
      DVE:  s = (bank > theta)                 (f32r {0,1}, 1024 wide)
      DMA:  s -> spk[b,t] in SBUF layout; host un-layouts to (B,T,C,H,W).

Conv precision: 3-term fp16 split. x ~ a + b with a=fp16(x), b=fp16-ish
remainder; W ~ Wa + Wb. x@W = a@Wa + (x-a)@Wa + a@(W-Wa) + O(2^-24).
The small terms are pre-scaled by 64 (operands) / 1/64 (weights) so every
fp16 operand stays in normal range (subnormal-flush-proof); products are
unchanged. Rows: rhs=[a; 64(x-a); a/64], lhsT=[Wa; Wa/64; 64(W-Wa)], K=108.

The rhs is im2col'ed on the HOST into the exact SBUF tile layout
(row k = j*6+kw*2+cin with j=r+kh, col n = g*64+w, h=4g+r), so each frame
loads with a single DMA instruction; DMA_DIRECT2D dispatch (~0.7us each on
the issuing engine queue) was the baseline bottleneck. Spikes are written
to DRAM in the SBUF layout (one DMA per (b,t)) and un-layouted on host.
"""

import glob
import json
import os
import subprocess
import sys
import tempfile
import time

sys.path.insert(0, "/opt/trn_rl_repo")

import numpy as np

import concourse.bacc as bacc
import concourse.tile as tile
from concourse import mybir
from concourse.bass_utils import run_bass_kernel_spmd

F32 = mybir.dt.float32
F32R = mybir.dt.float32r
F16 = mybir.dt.float16

B, T, CIN, H, W = 16, 20, 2, 64, 64
COUT, KS = 32, 3
NC_ = 8
BLOC = B // NC_          # 2 batches per core
NF = BLOC * T            # 40 frames per core
EPS = 1e-5
K1 = 36                  # base im2col rows (6 row6 x 3 kw x 2 cin)
K3 = 108                 # 3-term rows
NPIX = 1024              # free size per frame (16 groups x 64 cols)

LAST_EXEC_NS = {}
LAST_INFO = {}
TRACE = bool(os.environ.get("KERNEL_TRACE"))


def _trunc12(a):
    return (np.ascontiguousarray(a).view(np.uint32) & np.uint32(0xFFFFF000)).view(
        np.float32
    )


def _w_block(w):
    """[36,128] weight block: k=(row6*6+kw*2+cin), m=(cout*4+r)."""
    wb = np.zeros((K1, 128), np.float64)
    for r in range(4):
        for kh in range(KS):
            k6 = r + kh
            for kw in range(KS):
                for cin in range(CIN):
                    wb[k6 * 6 + kw * 2 + cin, r::4] = w[:, cin, kh, kw]
    return wb


def _split3_rhs(r32):
    """[..., 36, n] f32 -> [..., 108, n] fp16: [a; 64(x-a); a/64]."""
    a = r32.astype(np.float16)
    a32 = a.astype(np.float32)
    bs = ((r32 - a32) * np.float32(64.0)).astype(np.float16)
    as_ = (a32 / np.float32(64.0)).astype(np.float16)
    return np.concatenate([a, bs, as_], axis=-2)


def _split3_w(wb32):
    """[36,128] f32 -> [108,128] fp16: [Wa; Wa/64; 64(W-Wa)]."""
    wa = wb32.astype(np.float16)
    wa32 = wa.astype(np.float32)
    was = (wa32 / np.float32(64.0)).astype(np.float16)
    wbs = ((wb32 - wa32) * np.float32(64.0)).astype(np.float16)
    return np.concatenate([wa, was, wbs], axis=0)


def _phase1():
    nc = bacc.Bacc("TRN2", target_bir_lowering=False, debug=False, num_devices=NC_)
    rhs_d = nc.dram_tensor("rhs", [BLOC, T, K3, NPIX], F16, kind="ExternalInput")
    w_d = nc.dram_tensor("w1", [K3, 128], F16, kind="ExternalInput")
    st_d = nc.dram_tensor("stats", [128, NF * 12], F32, kind="ExternalOutput")

    with tile.TileContext(nc) as tc:
        with (
            tc.tile_pool(name="res", bufs=1) as res,
            tc.tile_pool(name="rp", bufs=8) as rp,
            tc.tile_pool(name="psum", bufs=4, space="PSUM") as psum,
        ):
            w1 = res.tile([K3, 128], F16)
            nc.sync.dma_start(w1[:], w_d[:])
            statsbuf = res.tile([128, NF * 12], F32)
            for f in range(NF):
                b, t = divmod(f, T)
                r = rp.tile([K3, NPIX], F16, name=f"r{f}", tag="r")
                eng = nc.sync if f % 2 == 0 else nc.scalar
                eng.dma_start(r[:], rhs_d[b, t])
                acc = psum.tile([128, NPIX], F32)
                for hf in range(2):
                    cols = slice(hf * 512, hf * 512 + 512)
                    nc.tensor.matmul(
                        acc[:, cols], w1[:], r[:, cols], start=True, stop=True
                    )
                for hf in range(2):
                    nc.vector.bn_stats(
                        statsbuf[:, f * 12 + hf * 6 : f * 12 + hf * 6 + 6],
                        acc[:, hf * 512 : hf * 512 + 512],
                    )
            nc.sync.dma_start(st_d[:], statsbuf[:])
    nc.compile()
    return nc


def _phase2():
    nc = bacc.Bacc("TRN2", target_bir_lowering=False, debug=False, num_devices=NC_)
    rhs_d = nc.dram_tensor("rhs", [BLOC, T, K3, NPIX], F16, kind="ExternalInput")
    w_d = nc.dram_tensor("w2", [K3, 128], F16, kind="ExternalInput")
    ni_d = nc.dram_tensor("negI", [128, 128], F32R, kind="ExternalInput")
    bi_d = nc.dram_tensor("bias", [128, 1], F32, kind="ExternalInput")
    vi_d = nc.dram_tensor("vinit", [BLOC, 128, NPIX], F32, kind="ExternalInput")
    s0_d = nc.dram_tensor("sinit", [BLOC, 128, NPIX], F32R, kind="ExternalInput")
    out_d = nc.dram_tensor("spk", [BLOC, T, 128, NPIX], F32, kind="ExternalOutput")

    BETA = _phase2.beta
    THETA = _phase2.theta

    with tile.TileContext(nc) as tc:
        with (
            tc.tile_pool(name="res", bufs=1) as res,
            tc.tile_pool(name="sp", bufs=6) as sp,
            tc.tile_pool(name="psum", bufs=1, space="PSUM") as psum,
        ):
            w2 = res.tile([K3, 128], F16)
            nc.sync.dma_start(w2[:], w_d[:])
            negI = res.tile([128, 128], F32R)
            nc.sync.dma_start(negI[:], ni_d[:])
            bias = res.tile([128, 1], F32)
            nc.sync.dma_start(bias[:], bi_d[:])
            vinit = res.tile([128, BLOC * NPIX], F32)
            sinit = res.tile([128, BLOC * NPIX], F32R)
            for b in range(BLOC):
                nc.sync.dma_start(vinit[:, b * NPIX : (b + 1) * NPIX], vi_d[b])
                nc.sync.dma_start(sinit[:, b * NPIX : (b + 1) * NPIX], s0_d[b])

            # rhs tiles, all SBUF resident; DMAs issued in consumption order
            rtiles = {}
            for t in range(T):
                for b in range(BLOC):
                    f = b * T + t
                    r = res.tile([K3, NPIX], F16, name=f"r{f}")
                    nc.sync.dma_start(r[:], rhs_d[b, t])
                    rtiles[f] = r

            banks = [
                [psum.tile([128, NPIX], F32, name=f"bk{b}_{par}") for par in range(2)]
                for b in range(BLOC)
            ]
            zl = res.tile([1, 128], F32R)
            nc.vector.memset(zl[:].bitcast(F32), 0.0)
            zr = res.tile([1, 512], F32R)
            nc.vector.memset(zr[:].bitcast(F32), 0.0)
            for b in range(BLOC):
                for par in range(2):
                    for hf in range(2):
                        cols = slice(hf * 512, hf * 512 + 512)
                        nc.tensor.matmul(
                            banks[b][par][:, cols], zl[:], zr[:], start=True, stop=True
                        )

            s_prev = {
                b: sinit[:, b * NPIX : (b + 1) * NPIX] for b in range(BLOC)
            }
            for t in range(T):
                cur = [banks[b][t % 2] for b in range(BLOC)]
                for b in range(BLOC):
                    if t == 0:
                        vsrc = vinit[:, b * NPIX : (b + 1) * NPIX]
                    else:
                        vsrc = banks[b][(t + 1) % 2][:]
                    nc.scalar.activation(
                        cur[b][:], vsrc,
                        mybir.ActivationFunctionType.Identity,
                        bias=bias[:], scale=BETA,
                    )
                for b in range(BLOC):
                    for hf in range(2):
                        cols = slice(hf * 512, hf * 512 + 512)
                        nc.tensor.matmul(
                            cur[b][:, cols], negI[:], s_prev[b][:, cols],
                            start=False, stop=True, skip_group_check=True,
                        )
                for b in range(BLOC):
                    f = b * T + t
                    for hf in range(2):
                        cols = slice(hf * 512, hf * 512 + 512)
                        nc.tensor.matmul(
                            cur[b][:, cols], w2[:], rtiles[f][:, cols],
                            start=False, stop=True, skip_group_check=True,
                        )
                for b in range(BLOC):
                    s = sp.tile([128, NPIX], F32R, name=f"s{t}_{b}", tag="s")
                    nc.vector.tensor_scalar(
                        out=s[:], in0=cur[b][:], scalar1=THETA, scalar2=None,
                        op0=mybir.AluOpType.is_gt,
                    )
                    nc.sync.dma_start(out_d[b, t], s[:].bitcast(F32))
                    s_prev[b] = s[:]
    nc.compile()
    return nc


def _get_hook():
    try:
        from trn_agent_boot.trn_boot import _ntff_profile_via_ctypes

        return _ntff_profile_via_ctypes("/opt/axon/libaxon_pjrt.so")
    except Exception:
        return None


def _ntff_exec_ns(d):
    """Max device exec time (ns) over all profiled cores via neuron-profile."""
    try:
        neffs = glob.glob(os.path.join(d, "*.neff"))
        ntffs = sorted(glob.glob(os.path.join(d, "*device*.ntff")))
        if not neffs or not ntffs:
            return None
        best = None
        only0 = not os.environ.get("KERNEL_TRACE_ALL")
        for ntff in ntffs:
            if only0 and "device000000" not in ntff:
                continue
            jf = ntff + ".json"
            subprocess.check_call(
                ["neuron-profile", "view", "--ignore-nc-buf-usage",
                 "-s", ntff, "-n", neffs[0], "--output-format=json",
                 f"--output-file={jf}", "--ignore-dma-trace"],
                cwd=d, stdout=subprocess.DEVNULL, stderr=subprocess.DEVNULL,
            )
            with open(jf) as fh:
                summ = json.load(fh)["summary"][0]
            ns = int(float(summ["total_time"]) * 1e9)
            if best is None or ns > best:
                best = ns
        return best
    except Exception:
        return None


def _run(nc, in_maps, label):
    t0 = time.time()
    if TRACE:
        hook = _get_hook()
        if hook is not None:
            d = tempfile.mkdtemp(prefix=f"ntff_{label}_")
            ctx = None
            try:
                ctx = hook(d, None)
                ctx.__enter__()
            except Exception:
                ctx = None
            if ctx is not None:
                try:
                    r = run_bass_kernel_spmd(nc, in_maps, core_ids=list(range(NC_)))
                finally:
                    try:
                        ctx.__exit__(None, None, None)
                    except Exception:
                        pass
                wall = (time.time() - t0) * 1e9
                ns = _ntff_exec_ns(d)
                LAST_EXEC_NS[label] = ns if ns is not None else wall
                LAST_INFO[label + "_wall_ns"] = wall
                LAST_INFO[label + "_trace_dir"] = d
                return r
    r = run_bass_kernel_spmd(nc, in_maps, core_ids=list(range(NC_)))
    LAST_EXEC_NS[label] = (time.time() - t0) * 1e9
    return r


def kernel(x, mem_init, conv_w, conv_b, bn_gamma, bn_bias, beta, threshold):
    x = np.asarray(x, np.float32)
    mem_init = np.asarray(mem_init, np.float32)
    conv_w = np.asarray(conv_w, np.float32)
    bn_gamma = np.asarray(bn_gamma, np.float32)
    bn_bias = np.asarray(bn_bias, np.float32)
    betac = float(np.clip(np.float32(beta), 0.0, 1.0))
    theta = float(np.float32(threshold))

    # ---- host prep: padded input -> im2col rhs in the SBUF tile layout
    xp = np.zeros((B, T, CIN, 66, 66), np.float32)
    xp[:, :, :, 1:65, 1:65] = x
    s_ = xp.strides
    # V[b,t,j,kw,cin,g,w] = xp[b,t,cin,4g+j,kw+w]
    v = np.lib.stride_tricks.as_strided(
        xp,
        (B, T, 6, KS, CIN, 16, 64),
        (s_[0], s_[1], s_[3], s_[4], s_[2], 4 * s_[3], s_[4]),
    )
    r32 = np.ascontiguousarray(v).reshape(B, T, K1, NPIX)
    rhs_all = np.ascontiguousarray(_split3_rhs(r32))  # [B,T,108,1024] fp16

    wb = _w_block(conv_w)  # [36,128] fp64
    w1 = _split3_w(wb.astype(np.float32))

    # ---- phase 1: per-channel stats of the conv output
    nc1 = _phase1()
    in_maps1 = [
        {"rhs": rhs_all[c * BLOC : (c + 1) * BLOC], "w1": w1} for c in range(NC_)
    ]
    r1 = _run(nc1, in_maps1, "phase1")

    # ---- host: combine stats (each 6-tuple: [cnt,mean,M2, cnt,mean,M2])
    tot_s = np.zeros(COUT, np.float64)
    tot_q = np.zeros(COUT, np.float64)
    for c in range(NC_):
        st = r1.results[c]["stats"].astype(np.float64).reshape(128, NF * 2, 6)
        for half in (0, 3):
            cnt = st[:, :, half]
            mean = st[:, :, half + 1]
            m2 = st[:, :, half + 2]
            tot_s += (cnt * mean).reshape(32, 4, -1).sum(axis=(1, 2))
            tot_q += (m2 + cnt * mean * mean).reshape(32, 4, -1).sum(axis=(1, 2))
    n_tot = float(B * T * H * W)
    mu = tot_s / n_tot
    var = tot_q / n_tot - mu * mu
    gp = bn_gamma.astype(np.float64) / np.sqrt(var + EPS)
    # reference normalizes y=conv+cb, but cb cancels: b'' = bn_bias - gp*mu
    bpp = bn_bias.astype(np.float64) - gp * mu
    wb2 = (wb * np.repeat(gp, 4)[None, :]).astype(np.float32)
    w2 = _split3_w(wb2)

    bias128 = np.repeat(bpp, 4).astype(np.float32).reshape(128, 1)
    negI = _trunc12(-theta * np.eye(128, dtype=np.float32))

    def to_layout(a):
        # [B, C, H, W] -> [B, p=c*4+r, n=g*64+w] with h = 4g+r
        a = a.reshape(B, COUT, 16, 4, 64)
        return np.ascontiguousarray(a.transpose(0, 1, 3, 2, 4).reshape(B, 128, NPIX))

    v0 = to_layout(mem_init.astype(np.float32))
    s0 = to_layout((mem_init > theta).astype(np.float32))

    _phase2.beta = betac
    _phase2.theta = theta
    nc2 = _phase2()
    in_maps2 = [
        {
            "rhs": rhs_all[c * BLOC : (c + 1) * BLOC],
            "w2": w2, "negI": negI, "bias": bias128,
            "vinit": v0[c * BLOC : (c + 1) * BLOC],
            "sinit": s0[c * BLOC : (c + 1) * BLOC],
        }
        for c in range(NC_)
    ]
    r2 = _run(nc2, in_maps2, "phase2")

    out = np.concatenate([r2.results[c]["spk"] for c in range(NC_)], axis=0)
    # [B,T,128,1024] -> [B,T,C,H,W]: p=c*4+r, n=g*64+w, h=4g+r
    out = (
        out.reshape(B, T, COUT, 4, 16, 64)
        .transpose(0, 1, 2, 4, 3, 5)
        .reshape(B, T, COUT, H, W)
    )
    return np.ascontiguousarray(out).astype(np.float32)


# revision 13
# speedup vs baseline: 50355.8120x; 1.0510x over previous
"""ConvSpikingBlock Trainium2 kernel (8 NeuronCores, data-parallel over batch).

Per core (2 of 16 batches, 40 frames):
  phase 1 NEFF: conv via ONE fp16 matmul per frame-half (3-term split, see
    below) -> bn_stats per half -> raw stats out; host combines stats across
    cores in fp64 and folds the BN affine into the conv weights.
  phase 2 NEFF: conv with folded weights accumulates onto a PSUM-resident
    membrane; per step t, per batch b:
      ACT:  bank = beta * v_prev + b''         (per-partition bias, 1024 wide)
      PE :  bank += (-theta I) @ s_prev        (f32r reset, per 512 half)
      PE :  bank += W3.T @ rhs3[f]             (fp16 conv, per 512 half)
      DVE:  s = (bank > theta)                 (f32r {0,1}, 1024 wide)
      DMA:  s -> spk[b,t] in SBUF layout; host un-layouts to (B,T,C,H,W).

Conv precision: 3-term fp16 split. x ~ a + b with a=fp16(x), b=fp16-ish
remainder; W ~ Wa + Wb. x@W = a@Wa + (x-a)@Wa + a@(W-Wa) + O(2^-24).
The small terms are pre-scaled by 64 (operands) / 1/64 (weights) so every
fp16 operand stays in normal range (subnormal-flush-proof); products are
unchanged. Rows: rhs=[a; 64(x-a); a/64], lhsT=[Wa; Wa/64; 64(W-Wa)], K=108.

The rhs is im2col'ed on the HOST into the exact SBUF tile layout
(row k = j*6+kw*2+cin with j=r+kh, col n = g*64+w, h=4g+r), so each frame
loads with a single DMA instruction; DMA_DIRECT2D dispatch (~0.7us each on
the issuing engine queue) was the baseline bottleneck. Spikes are written
to DRAM in the SBUF layout (one DMA per (b,t)) and un-layouted on host.
"""

import glob
import json
import os
import subprocess
import sys
import tempfile
import time

sys.path.insert(0, "/opt/trn_rl_repo")

import ml_dtypes
import numpy as np

import concourse.bacc as bacc
import concourse.tile as tile
from concourse import mybir
from concourse.bass_utils import run_bass_kernel_spmd

F32 = mybir.dt.float32
F32R = mybir.dt.float32r
F16 = mybir.dt.float16
BF16 = mybir.dt.bfloat16

B, T, CIN, H, W = 16, 20, 2, 64, 64
COUT, KS = 32, 3
NC_ = 8
BLOC = B // NC_          # 2 batches per core
NF = BLOC * T            # 40 frames per core
EPS = 1e-5
K1 = 36                  # base im2col rows (6 row6 x 3 kw x 2 cin)
K3 = 108                 # 3-term rows
NPIX = 1024              # free size per frame (16 groups x 64 cols)

LAST_EXEC_NS = {}
LAST_INFO = {}
TRACE = bool(os.environ.get("KERNEL_TRACE"))


def _trunc12(a):
    return (np.ascontiguousarray(a).view(np.uint32) & np.uint32(0xFFFFF000)).view(
        np.float32
    )


def _w_block(w):
    """[36,128] weight block: k=(row6*6+kw*2+cin), m=(cout*4+r)."""
    wb = np.zeros((K1, 128), np.float64)
    for r in range(4):
        for kh in range(KS):
            k6 = r + kh
            for kw in range(KS):
                for cin in range(CIN):
                    wb[k6 * 6 + kw * 2 + cin, r::4] = w[:, cin, kh, kw]
    return wb


def _split3_rhs(r32):
    """[..., 36, n] f32 -> [..., 108, n] fp16: [a; 64(x-a); a/64]."""
    a = r32.astype(np.float16)
    a32 = a.astype(np.float32)
    bs = ((r32 - a32) * np.float32(64.0)).astype(np.float16)
    as_ = (a32 / np.float32(64.0)).astype(np.float16)
    return np.concatenate([a, bs, as_], axis=-2)


def _split3_w(wb32):
    """[36,128] f32 -> [108,128] fp16: [Wa; Wa/64; 64(W-Wa)]."""
    wa = wb32.astype(np.float16)
    wa32 = wa.astype(np.float32)
    was = (wa32 / np.float32(64.0)).astype(np.float16)
    wbs = ((wb32 - wa32) * np.float32(64.0)).astype(np.float16)
    return np.concatenate([wa, was, wbs], axis=0)


def _phase1():
    nc = bacc.Bacc("TRN2", target_bir_lowering=False, debug=False, num_devices=NC_)
    rhs_d = nc.dram_tensor("rhs", [BLOC, T, K3, NPIX], F16, kind="ExternalInput")
    w_d = nc.dram_tensor("w1", [K3, 128], F16, kind="ExternalInput")
    st_d = nc.dram_tensor("stats", [128, NF * 12], F32, kind="ExternalOutput")

    with tile.TileContext(nc) as tc:
        with (
            tc.tile_pool(name="res", bufs=1) as res,
            tc.tile_pool(name="rp", bufs=8) as rp,
            tc.tile_pool(name="psum", bufs=4, space="PSUM") as psum,
        ):
            w1 = res.tile([K3, 128], F16)
            nc.sync.dma_start(w1[:], w_d[:])
            statsbuf = res.tile([128, NF * 12], F32)
            for f in range(NF):
                b, t = divmod(f, T)
                r = rp.tile([K3, NPIX], F16, name=f"r{f}", tag="r")
                eng = nc.sync if f % 2 == 0 else nc.scalar
                eng.dma_start(r[:], rhs_d[b, t])
                acc = psum.tile([128, NPIX], F32)
                for hf in range(2):
                    cols = slice(hf * 512, hf * 512 + 512)
                    nc.tensor.matmul(
                        acc[:, cols], w1[:], r[:, cols], start=True, stop=True
                    )
                for hf in range(2):
                    nc.vector.bn_stats(
                        statsbuf[:, f * 12 + hf * 6 : f * 12 + hf * 6 + 6],
                        acc[:, hf * 512 : hf * 512 + 512],
                    )
            nc.sync.dma_start(st_d[:], statsbuf[:])
    nc.compile()
    return nc


def _phase2():
    nc = bacc.Bacc("TRN2", target_bir_lowering=False, debug=False, num_devices=NC_)
    rhs_d = nc.dram_tensor("rhs", [BLOC, T, K3, NPIX], F16, kind="ExternalInput")
    w_d = nc.dram_tensor("w2", [K3, 128], F16, kind="ExternalInput")
    ni_d = nc.dram_tensor("negI", [128, 128], BF16, kind="ExternalInput")
    bi_d = nc.dram_tensor("bias", [128, 1], F32, kind="ExternalInput")
    vi_d = nc.dram_tensor("vinit", [BLOC, 128, NPIX], F32, kind="ExternalInput")
    s0_d = nc.dram_tensor("sinit", [BLOC, 128, NPIX], BF16, kind="ExternalInput")
    out_d = nc.dram_tensor("spk", [BLOC, T, 128, NPIX], BF16, kind="ExternalOutput")

    BETA = _phase2.beta
    THETA = _phase2.theta

    with tile.TileContext(nc) as tc:
        with (
            tc.tile_pool(name="res", bufs=1) as res,
            tc.tile_pool(name="sp", bufs=6) as sp,
            tc.tile_pool(name="psum", bufs=1, space="PSUM") as psum,
        ):
            w2 = res.tile([K3, 128], F16)
            nc.sync.dma_start(w2[:], w_d[:])
            negI = res.tile([128, 128], BF16)
            nc.sync.dma_start(negI[:], ni_d[:])
            bias = res.tile([128, 1], F32)
            nc.sync.dma_start(bias[:], bi_d[:])
            vinit = res.tile([128, BLOC * NPIX], F32)
            sinit = res.tile([128, BLOC * NPIX], BF16)
            for b in range(BLOC):
                nc.sync.dma_start(vinit[:, b * NPIX : (b + 1) * NPIX], vi_d[b])
                nc.sync.dma_start(sinit[:, b * NPIX : (b + 1) * NPIX], s0_d[b])

            # rhs tiles, all SBUF resident; DMAs issued in consumption order
            rtiles = {}
            for t in range(T):
                for b in range(BLOC):
                    f = b * T + t
                    r = res.tile([K3, NPIX], F16, name=f"r{f}")
                    nc.sync.dma_start(r[:], rhs_d[b, t])
                    rtiles[f] = r

            banks = [
                [psum.tile([128, NPIX], F32, name=f"bk{b}_{par}") for par in range(2)]
                for b in range(BLOC)
            ]

            s_prev = {
                b: sinit[:, b * NPIX : (b + 1) * NPIX] for b in range(BLOC)
            }
            for t in range(T):
                cur = [banks[b][t % 2] for b in range(BLOC)]
                for b in range(BLOC):
                    if t == 0:
                        vsrc = vinit[:, b * NPIX : (b + 1) * NPIX]
                    else:
                        vsrc = banks[b][(t + 1) % 2][:]
                    nc.scalar.activation(
                        cur[b][:], vsrc,
                        mybir.ActivationFunctionType.Identity,
                        bias=bias[:], scale=BETA,
                    )
                for b in range(BLOC):
                    for hf in range(2):
                        cols = slice(hf * 512, hf * 512 + 512)
                        nc.tensor.matmul(
                            cur[b][:, cols], negI[:], s_prev[b][:, cols],
                            start=False, stop=True, skip_group_check=True,
                        )
                for b in range(BLOC):
                    f = b * T + t
                    for hf in range(2):
                        cols = slice(hf * 512, hf * 512 + 512)
                        nc.tensor.matmul(
                            cur[b][:, cols], w2[:], rtiles[f][:, cols],
                            start=False, stop=True, skip_group_check=True,
                        )
                for b in range(BLOC):
                    s = sp.tile([128, NPIX], BF16, name=f"s{t}_{b}", tag="s")
                    nc.vector.tensor_scalar(
                        out=s[:], in0=cur[b][:], scalar1=THETA, scalar2=None,
                        op0=mybir.AluOpType.is_gt,
                    )
                    nc.sync.dma_start(out_d[b, t], s[:])
                    s_prev[b] = s[:]
    nc.compile()
    return nc


def _get_hook():
    try:
        from trn_agent_boot.trn_boot import _ntff_profile_via_ctypes

        return _ntff_profile_via_ctypes("/opt/axon/libaxon_pjrt.so")
    except Exception:
        return None


def _ntff_exec_ns(d):
    """Max device exec time (ns) over all profiled cores via neuron-profile."""
    try:
        neffs = glob.glob(os.path.join(d, "*.neff"))
        ntffs = sorted(glob.glob(os.path.join(d, "*device*.ntff")))
        if not neffs or not ntffs:
            return None
        best = None
        only0 = not os.environ.get("KERNEL_TRACE_ALL")
        for ntff in ntffs:
            if only0 and "device000000" not in ntff:
                continue
            jf = ntff + ".json"
            subprocess.check_call(
                ["neuron-profile", "view", "--ignore-nc-buf-usage",
                 "-s", ntff, "-n", neffs[0], "--output-format=json",
                 f"--output-file={jf}", "--ignore-dma-trace"],
                cwd=d, stdout=subprocess.DEVNULL, stderr=subprocess.DEVNULL,
            )
            with open(jf) as fh:
                summ = json.load(fh)["summary"][0]
            ns = int(float(summ["total_time"]) * 1e9)
            if best is None or ns > best:
                best = ns
        return best
    except Exception:
        return None


def _run(nc, in_maps, label):
    t0 = time.time()
    if TRACE:
        hook = _get_hook()
        if hook is not None:
            d = tempfile.mkdtemp(prefix=f"ntff_{label}_")
            ctx = None
            try:
                ctx = hook(d, None)
                ctx.__enter__()
            except Exception:
                ctx = None
            if ctx is not None:
                try:
                    r = run_bass_kernel_spmd(nc, in_maps, core_ids=list(range(NC_)))
                finally:
                    try:
                        ctx.__exit__(None, None, None)
                    except Exception:
                        pass
                wall = (time.time() - t0) * 1e9
                ns = _ntff_exec_ns(d)
                LAST_EXEC_NS[label] = ns if ns is not None else wall
                LAST_INFO[label + "_wall_ns"] = wall
                LAST_INFO[label + "_trace_dir"] = d
                return r
    r = run_bass_kernel_spmd(nc, in_maps, core_ids=list(range(NC_)))
    LAST_EXEC_NS[label] = (time.time() - t0) * 1e9
    return r


def kernel(x, mem_init, conv_w, conv_b, bn_gamma, bn_bias, beta, threshold):
    x = np.asarray(x, np.float32)
    mem_init = np.asarray(mem_init, np.float32)
    conv_w = np.asarray(conv_w, np.float32)
    bn_gamma = np.asarray(bn_gamma, np.float32)
    bn_bias = np.asarray(bn_bias, np.float32)
    betac = float(np.clip(np.float32(beta), 0.0, 1.0))
    theta = float(np.float32(threshold))

    # ---- host prep: padded input -> im2col rhs in the SBUF tile layout
    xp = np.zeros((B, T, CIN, 66, 66), np.float32)
    xp[:, :, :, 1:65, 1:65] = x
    s_ = xp.strides
    # V[b,t,j,kw,cin,g,w] = xp[b,t,cin,4g+j,kw+w]
    v = np.lib.stride_tricks.as_strided(
        xp,
        (B, T, 6, KS, CIN, 16, 64),
        (s_[0], s_[1], s_[3], s_[4], s_[2], 4 * s_[3], s_[4]),
    )
    r32 = np.ascontiguousarray(v).reshape(B, T, K1, NPIX)
    rhs_all = np.ascontiguousarray(_split3_rhs(r32))  # [B,T,108,1024] fp16

    wb = _w_block(conv_w)  # [36,128] fp64
    w1 = _split3_w(wb.astype(np.float32))

    # ---- phase 1: per-channel stats of the conv output
    nc1 = _phase1()
    in_maps1 = [
        {"rhs": rhs_all[c * BLOC : (c + 1) * BLOC], "w1": w1} for c in range(NC_)
    ]
    r1 = _run(nc1, in_maps1, "phase1")

    # ---- host: combine stats (each 6-tuple: [cnt,mean,M2, cnt,mean,M2])
    tot_s = np.zeros(COUT, np.float64)
    tot_q = np.zeros(COUT, np.float64)
    for c in range(NC_):
        st = r1.results[c]["stats"].astype(np.float64).reshape(128, NF * 2, 6)
        for half in (0, 3):
            cnt = st[:, :, half]
            mean = st[:, :, half + 1]
            m2 = st[:, :, half + 2]
            tot_s += (cnt * mean).reshape(32, 4, -1).sum(axis=(1, 2))
            tot_q += (m2 + cnt * mean * mean).reshape(32, 4, -1).sum(axis=(1, 2))
    n_tot = float(B * T * H * W)
    mu = tot_s / n_tot
    var = tot_q / n_tot - mu * mu
    gp = bn_gamma.astype(np.float64) / np.sqrt(var + EPS)
    # reference normalizes y=conv+cb, but cb cancels: b'' = bn_bias - gp*mu
    bpp = bn_bias.astype(np.float64) - gp * mu
    wb2 = (wb * np.repeat(gp, 4)[None, :]).astype(np.float32)
    w2 = _split3_w(wb2)

    bias128 = np.repeat(bpp, 4).astype(np.float32).reshape(128, 1)
    negI = (-theta * np.eye(128, dtype=np.float32)).astype(ml_dtypes.bfloat16)

    def to_layout(a):
        # [B, C, H, W] -> [B, p=c*4+r, n=g*64+w] with h = 4g+r
        a = a.reshape(B, COUT, 16, 4, 64)
        return np.ascontiguousarray(a.transpose(0, 1, 3, 2, 4).reshape(B, 128, NPIX))

    v0 = to_layout(mem_init.astype(np.float32))
    s0 = to_layout((mem_init > theta).astype(np.float32)).astype(ml_dtypes.bfloat16)

    _phase2.beta = betac
    _phase2.theta = theta
    nc2 = _phase2()
    in_maps2 = [
        {
            "rhs": rhs_all[c * BLOC : (c + 1) * BLOC],
            "w2": w2, "negI": negI, "bias": bias128,
            "vinit": v0[c * BLOC : (c + 1) * BLOC],
            "sinit": s0[c * BLOC : (c + 1) * BLOC],
        }
        for c in range(NC_)
    ]
    r2 = _run(nc2, in_maps2, "phase2")

    out = np.concatenate([r2.results[c]["spk"] for c in range(NC_)], axis=0)
    # [B,T,128,1024] bf16 {0,1} -> [B,T,C,H,W] f32: p=c*4+r, n=g*64+w, h=4g+r
    out = (
        out.reshape(B, T, COUT, 4, 16, 64)
        .transpose(0, 1, 2, 4, 3, 5)
        .reshape(B, T, COUT, H, W)
    )
    return np.ascontiguousarray(out).astype(np.float32)


# revision 17
# speedup vs baseline: 51212.0051x; 1.0170x over previous
"""ConvSpikingBlock Trainium2 kernel (8 NeuronCores, data-parallel over batch).

Per core (2 of 16 batches, 40 frames):
  phase 1 NEFF: conv via ONE fp16 matmul per frame-half (3-term split, see
    below) -> bn_stats per half -> raw stats out; host combines stats across
    cores in fp64 and folds the BN affine into the conv weights.
  phase 2 NEFF: conv with folded weights accumulates onto a PSUM-resident
    membrane; per step t, per batch b:
      ACT:  bank = beta * v_prev + b''         (per-partition bias, 1024 wide)
      PE :  bank += (-theta I) @ s_prev        (f32r reset, per 512 half)
      PE :  bank += W3.T @ rhs3[f]             (fp16 conv, per 512 half)
      DVE:  s = (bank > theta)                 (f32r {0,1}, 1024 wide)
      DMA:  s -> spk[b,t] in SBUF layout; host un-layouts to (B,T,C,H,W).

Conv precision: 3-term fp16 split. x ~ a + b with a=fp16(x), b=fp16-ish
remainder; W ~ Wa + Wb. x@W = a@Wa + (x-a)@Wa + a@(W-Wa) + O(2^-24).
The small terms are pre-scaled by 64 (operands) / 1/64 (weights) so every
fp16 operand stays in normal range (subnormal-flush-proof); products are
unchanged. Rows: rhs=[a; 64(x-a); a/64], lhsT=[Wa; Wa/64; 64(W-Wa)], K=108.

The rhs is im2col'ed on the HOST into the exact SBUF tile layout
(row k = j*6+kw*2+cin with j=r+kh, col n = g*64+w, h=4g+r), so each frame
loads with a single DMA instruction; DMA_DIRECT2D dispatch (~0.7us each on
the issuing engine queue) was the baseline bottleneck. Spikes are written
to DRAM in the SBUF layout (one DMA per (b,t)) and un-layouted on host.
"""

import glob
import json
import os
import subprocess
import sys
import tempfile
import time

sys.path.insert(0, "/opt/trn_rl_repo")

import ml_dtypes
import numpy as np

import concourse.bacc as bacc
import concourse.tile as tile
from concourse import mybir
from concourse.bass_utils import run_bass_kernel_spmd

F32 = mybir.dt.float32
F32R = mybir.dt.float32r
F16 = mybir.dt.float16
BF16 = mybir.dt.bfloat16

B, T, CIN, H, W = 16, 20, 2, 64, 64
COUT, KS = 32, 3
NC_ = 8
BLOC = B // NC_          # 2 batches per core
NF = BLOC * T            # 40 frames per core
EPS = 1e-5
K1 = 36                  # base im2col rows (6 row6 x 3 kw x 2 cin)
K3 = 108                 # 3-term rows
NPIX = 1024              # free size per frame (16 groups x 64 cols)

LAST_EXEC_NS = {}
LAST_INFO = {}
TRACE = bool(os.environ.get("KERNEL_TRACE"))


def _trunc12(a):
    return (np.ascontiguousarray(a).view(np.uint32) & np.uint32(0xFFFFF000)).view(
        np.float32
    )


def _w_block(w):
    """[36,128] weight block: k=(row6*6+kw*2+cin), m=(cout*4+r)."""
    wb = np.zeros((K1, 128), np.float64)
    for r in range(4):
        for kh in range(KS):
            k6 = r + kh
            for kw in range(KS):
                for cin in range(CIN):
                    wb[k6 * 6 + kw * 2 + cin, r::4] = w[:, cin, kh, kw]
    return wb


def _split3_rhs(r32):
    """[..., 36, n] f32 -> [..., 108, n] fp16: [a; 64(x-a); a/64]."""
    a = r32.astype(np.float16)
    a32 = a.astype(np.float32)
    bs = ((r32 - a32) * np.float32(64.0)).astype(np.float16)
    as_ = (a32 / np.float32(64.0)).astype(np.float16)
    return np.concatenate([a, bs, as_], axis=-2)


def _split3_w(wb32):
    """[36,128] f32 -> [108,128] fp16: [Wa; Wa/64; 64(W-Wa)]."""
    wa = wb32.astype(np.float16)
    wa32 = wa.astype(np.float32)
    was = (wa32 / np.float32(64.0)).astype(np.float16)
    wbs = ((wb32 - wa32) * np.float32(64.0)).astype(np.float16)
    return np.concatenate([wa, was, wbs], axis=0)


# frames whose stats go through scalar-engine Sum/SumSq instead of DVE bn_stats
SC_FRAMES = frozenset(f for f in range(NF) if f % 4 == 3)
N_SC = len(SC_FRAMES)
N_DVE = NF - N_SC


def _phase1():
    nc = bacc.Bacc("TRN2", target_bir_lowering=False, debug=False, num_devices=NC_)
    rhs_d = nc.dram_tensor("rhs", [BLOC, T, K3, NPIX], F16, kind="ExternalInput")
    w_d = nc.dram_tensor("w1", [K3, 128], F16, kind="ExternalInput")
    st_d = nc.dram_tensor("stats", [128, N_DVE * 12], F32, kind="ExternalOutput")
    ss_d = nc.dram_tensor("ssum", [128, N_SC * 2], F32, kind="ExternalOutput")

    with tile.TileContext(nc) as tc:
        with (
            tc.tile_pool(name="res", bufs=1) as res,
            tc.tile_pool(name="rp", bufs=8) as rp,
            tc.tile_pool(name="scr", bufs=2) as scr,
            tc.tile_pool(name="psum", bufs=4, space="PSUM") as psum,
        ):
            w1 = res.tile([K3, 128], F16)
            nc.sync.dma_start(w1[:], w_d[:])
            statsbuf = res.tile([128, N_DVE * 12], F32)
            ssumbuf = res.tile([128, N_SC * 2], F32)
            dve_i = 0
            sc_i = 0
            for f in range(NF):
                b, t = divmod(f, T)
                r = rp.tile([K3, NPIX], F16, name=f"r{f}", tag="r")
                eng = nc.sync if f % 2 == 0 else nc.scalar
                if f < 2:
                    # split the first loads so the pipeline starts sooner
                    eng.dma_start(r[:, 0:512], rhs_d[b, t][:, 0:512])
                    eng.dma_start(r[:, 512:1024], rhs_d[b, t][:, 512:1024])
                else:
                    eng.dma_start(r[:], rhs_d[b, t])
                acc = psum.tile([128, NPIX], F32)
                for hf in range(2):
                    cols = slice(hf * 512, hf * 512 + 512)
                    nc.tensor.matmul(
                        acc[:, cols], w1[:], r[:, cols], start=True, stop=True
                    )
                if f in SC_FRAMES:
                    scratch = scr.tile([128, NPIX], F32, name=f"scr{f}", tag="scr")
                    nc.scalar.activation(
                        scratch[:], acc[:],
                        mybir.ActivationFunctionType.Identity,
                        accum_out=ssumbuf[:, sc_i * 2 : sc_i * 2 + 1],
                    )
                    nc.scalar.activation(
                        scratch[:], acc[:],
                        mybir.ActivationFunctionType.Square,
                        accum_out=ssumbuf[:, sc_i * 2 + 1 : sc_i * 2 + 2],
                    )
                    sc_i += 1
                else:
                    for hf in range(2):
                        nc.vector.bn_stats(
                            statsbuf[:, dve_i * 12 + hf * 6 : dve_i * 12 + hf * 6 + 6],
                            acc[:, hf * 512 : hf * 512 + 512],
                        )
                    dve_i += 1
            nc.sync.dma_start(st_d[:], statsbuf[:])
            nc.sync.dma_start(ss_d[:], ssumbuf[:])
    nc.compile()
    return nc


def _phase2():
    zi = _phase2.zero_init
    nc = bacc.Bacc("TRN2", target_bir_lowering=False, debug=False, num_devices=NC_)
    rhs_d = nc.dram_tensor("rhs", [BLOC, T, K3, NPIX], F16, kind="ExternalInput")
    w_d = nc.dram_tensor("w2", [K3, 128], F16, kind="ExternalInput")
    ni_d = nc.dram_tensor("negI", [128, 128], BF16, kind="ExternalInput")
    bi_d = nc.dram_tensor("bias", [128, 1], F32, kind="ExternalInput")
    if not zi:
        vi_d = nc.dram_tensor("vinit", [BLOC, 128, NPIX], F32, kind="ExternalInput")
        s0_d = nc.dram_tensor("sinit", [BLOC, 128, NPIX], BF16, kind="ExternalInput")
    out_d = nc.dram_tensor("spk", [BLOC, T, 128, NPIX], BF16, kind="ExternalOutput")

    BETA = _phase2.beta
    THETA = _phase2.theta

    with tile.TileContext(nc) as tc:
        with (
            tc.tile_pool(name="res", bufs=1) as res,
            tc.tile_pool(name="sp", bufs=6) as sp,
            tc.tile_pool(name="psum", bufs=1, space="PSUM") as psum,
        ):
            w2 = res.tile([K3, 128], F16)
            nc.sync.dma_start(w2[:], w_d[:])
            negI = res.tile([128, 128], BF16)
            nc.sync.dma_start(negI[:], ni_d[:])
            bias = res.tile([128, 1], F32)
            nc.sync.dma_start(bias[:], bi_d[:])
            if not zi:
                vinit = res.tile([128, BLOC * NPIX], F32)
                sinit = res.tile([128, BLOC * NPIX], BF16)
                for b in range(BLOC):
                    nc.sync.dma_start(vinit[:, b * NPIX : (b + 1) * NPIX], vi_d[b])
                    nc.sync.dma_start(sinit[:, b * NPIX : (b + 1) * NPIX], s0_d[b])

            # rhs tiles, all SBUF resident; DMAs issued in consumption order
            rtiles = {}
            for t in range(T):
                for b in range(BLOC):
                    f = b * T + t
                    r = res.tile([K3, NPIX], F16, name=f"r{f}")
                    if t == 0:
                        nc.sync.dma_start(r[:, 0:512], rhs_d[b, t][:, 0:512])
                        nc.sync.dma_start(r[:, 512:1024], rhs_d[b, t][:, 512:1024])
                    else:
                        nc.sync.dma_start(r[:], rhs_d[b, t])
                    rtiles[f] = r

            banks = [
                [psum.tile([128, NPIX], F32, name=f"bk{b}_{par}") for par in range(2)]
                for b in range(BLOC)
            ]

            s_prev = {}
            if not zi:
                for b in range(BLOC):
                    s_prev[b] = sinit[:, b * NPIX : (b + 1) * NPIX]
            for t in range(T):
                cur = [banks[b][t % 2] for b in range(BLOC)]
                for b in range(BLOC):
                    if t == 0:
                        if zi:
                            # v_init == 0: bank = 0*garbage + bias
                            nc.scalar.activation(
                                cur[b][:], cur[b][:],
                                mybir.ActivationFunctionType.Identity,
                                bias=bias[:], scale=0.0,
                            )
                        else:
                            nc.scalar.activation(
                                cur[b][:], vinit[:, b * NPIX : (b + 1) * NPIX],
                                mybir.ActivationFunctionType.Identity,
                                bias=bias[:], scale=BETA,
                            )
                    else:
                        nc.scalar.activation(
                            cur[b][:], banks[b][(t + 1) % 2][:],
                            mybir.ActivationFunctionType.Identity,
                            bias=bias[:], scale=BETA,
                        )
                if t > 0 or not zi:
                    for b in range(BLOC):
                        for hf in range(2):
                            cols = slice(hf * 512, hf * 512 + 512)
                            nc.tensor.matmul(
                                cur[b][:, cols], negI[:], s_prev[b][:, cols],
                                start=False, stop=True, skip_group_check=True,
                            )
                for b in range(BLOC):
                    f = b * T + t
                    for hf in range(2):
                        cols = slice(hf * 512, hf * 512 + 512)
                        nc.tensor.matmul(
                            cur[b][:, cols], w2[:], rtiles[f][:, cols],
                            start=False, stop=True, skip_group_check=True,
                        )
                for b in range(BLOC):
                    s = sp.tile([128, NPIX], BF16, name=f"s{t}_{b}", tag="s")
                    nc.vector.tensor_scalar(
                        out=s[:], in0=cur[b][:], scalar1=THETA, scalar2=None,
                        op0=mybir.AluOpType.is_gt,
                    )
                    nc.sync.dma_start(out_d[b, t], s[:])
                    s_prev[b] = s[:]
    nc.compile()
    return nc


def _get_hook():
    try:
        from trn_agent_boot.trn_boot import _ntff_profile_via_ctypes

        return _ntff_profile_via_ctypes("/opt/axon/libaxon_pjrt.so")
    except Exception:
        return None


def _ntff_exec_ns(d):
    """Max device exec time (ns) over all profiled cores via neuron-profile."""
    try:
        neffs = glob.glob(os.path.join(d, "*.neff"))
        ntffs = sorted(glob.glob(os.path.join(d, "*device*.ntff")))
        if not neffs or not ntffs:
            return None
        best = None
        only0 = not os.environ.get("KERNEL_TRACE_ALL")
        for ntff in ntffs:
            if only0 and "device000000" not in ntff:
                continue
            jf = ntff + ".json"
            subprocess.check_call(
                ["neuron-profile", "view", "--ignore-nc-buf-usage",
                 "-s", ntff, "-n", neffs[0], "--output-format=json",
                 f"--output-file={jf}", "--ignore-dma-trace"],
                cwd=d, stdout=subprocess.DEVNULL, stderr=subprocess.DEVNULL,
            )
            with open(jf) as fh:
                summ = json.load(fh)["summary"][0]
            ns = int(float(summ["total_time"]) * 1e9)
            if best is None or ns > best:
                best = ns
        return best
    except Exception:
        return None


def _run(nc, in_maps, label):
    t0 = time.time()
    if TRACE:
        hook = _get_hook()
        if hook is not None:
            d = tempfile.mkdtemp(prefix=f"ntff_{label}_")
            ctx = None
            try:
                ctx = hook(d, None)
                ctx.__enter__()
            except Exception:
                ctx = None
            if ctx is not None:
                try:
                    r = run_bass_kernel_spmd(nc, in_maps, core_ids=list(range(NC_)))
                finally:
                    try:
                        ctx.__exit__(None, None, None)
                    except Exception:
                        pass
                wall = (time.time() - t0) * 1e9
                ns = _ntff_exec_ns(d)
                LAST_EXEC_NS[label] = ns if ns is not None else wall
                LAST_INFO[label + "_wall_ns"] = wall
                LAST_INFO[label + "_trace_dir"] = d
                return r
    r = run_bass_kernel_spmd(nc, in_maps, core_ids=list(range(NC_)))
    LAST_EXEC_NS[label] = (time.time() - t0) * 1e9
    return r


def kernel(x, mem_init, conv_w, conv_b, bn_gamma, bn_bias, beta, threshold):
    x = np.asarray(x, np.float32)
    mem_init = np.asarray(mem_init, np.float32)
    conv_w = np.asarray(conv_w, np.float32)
    bn_gamma = np.asarray(bn_gamma, np.float32)
    bn_bias = np.asarray(bn_bias, np.float32)
    betac = float(np.clip(np.float32(beta), 0.0, 1.0))
    theta = float(np.float32(threshold))

    # ---- host prep: padded input -> im2col rhs in the SBUF tile layout
    xp = np.zeros((B, T, CIN, 66, 66), np.float32)
    xp[:, :, :, 1:65, 1:65] = x
    s_ = xp.strides
    # V[b,t,j,kw,cin,g,w] = xp[b,t,cin,4g+j,kw+w]
    v = np.lib.stride_tricks.as_strided(
        xp,
        (B, T, 6, KS, CIN, 16, 64),
        (s_[0], s_[1], s_[3], s_[4], s_[2], 4 * s_[3], s_[4]),
    )
    r32 = np.ascontiguousarray(v).reshape(B, T, K1, NPIX)
    rhs_all = np.ascontiguousarray(_split3_rhs(r32))  # [B,T,108,1024] fp16

    wb = _w_block(conv_w)  # [36,128] fp64
    w1 = _split3_w(wb.astype(np.float32))

    # ---- phase 1: per-channel stats of the conv output
    nc1 = _phase1()
    in_maps1 = [
        {"rhs": rhs_all[c * BLOC : (c + 1) * BLOC], "w1": w1} for c in range(NC_)
    ]
    r1 = _run(nc1, in_maps1, "phase1")

    # ---- host: combine stats (each 6-tuple: [cnt,mean,M2, cnt,mean,M2])
    tot_s = np.zeros(COUT, np.float64)
    tot_q = np.zeros(COUT, np.float64)
    for c in range(NC_):
        st = r1.results[c]["stats"].astype(np.float64).reshape(128, N_DVE * 2, 6)
        for half in (0, 3):
            cnt = st[:, :, half]
            mean = st[:, :, half + 1]
            m2 = st[:, :, half + 2]
            tot_s += (cnt * mean).reshape(32, 4, -1).sum(axis=(1, 2))
            tot_q += (m2 + cnt * mean * mean).reshape(32, 4, -1).sum(axis=(1, 2))
        ss = r1.results[c]["ssum"].astype(np.float64).reshape(128, N_SC, 2)
        tot_s += ss[:, :, 0].reshape(32, 4, -1).sum(axis=(1, 2))
        tot_q += ss[:, :, 1].reshape(32, 4, -1).sum(axis=(1, 2))
    n_tot = float(B * T * H * W)
    mu = tot_s / n_tot
    var = tot_q / n_tot - mu * mu
    gp = bn_gamma.astype(np.float64) / np.sqrt(var + EPS)
    # reference normalizes y=conv+cb, but cb cancels: b'' = bn_bias - gp*mu
    bpp = bn_bias.astype(np.float64) - gp * mu
    wb2 = (wb * np.repeat(gp, 4)[None, :]).astype(np.float32)
    w2 = _split3_w(wb2)

    bias128 = np.repeat(bpp, 4).astype(np.float32).reshape(128, 1)
    negI = (-theta * np.eye(128, dtype=np.float32)).astype(ml_dtypes.bfloat16)

    def to_layout(a):
        # [B, C, H, W] -> [B, p=c*4+r, n=g*64+w] with h = 4g+r
        a = a.reshape(B, COUT, 16, 4, 64)
        return np.ascontiguousarray(a.transpose(0, 1, 3, 2, 4).reshape(B, 128, NPIX))

    v0 = to_layout(mem_init.astype(np.float32))
    s0 = to_layout((mem_init > theta).astype(np.float32)).astype(ml_dtypes.bfloat16)
    zero_init = bool((v0 == 0).all() and (s0 == 0).all())

    _phase2.beta = betac
    _phase2.theta = theta
    _phase2.zero_init = zero_init
    nc2 = _phase2()
    in_maps2 = []
    for c in range(NC_):
        m = {
            "rhs": rhs_all[c * BLOC : (c + 1) * BLOC],
            "w2": w2, "negI": negI, "bias": bias128,
        }
        if not zero_init:
            m["vinit"] = v0[c * BLOC : (c + 1) * BLOC]
            m["sinit"] = s0[c * BLOC : (c + 1) * BLOC]
        in_maps2.append(m)
    r2 = _run(nc2, in_maps2, "phase2")

    out = np.concatenate([r2.results[c]["spk"] for c in range(NC_)], axis=0)
    # [B,T,128,1024] bf16 {0,1} -> [B,T,C,H,W] f32: p=c*4+r, n=g*64+w, h=4g+r
    out = (
        out.reshape(B, T, COUT, 4, 16, 64)
        .transpose(0, 1, 2, 4, 3, 5)
        .reshape(B, T, COUT, H, W)
    )
    return np.ascontiguousarray(out).astype(np.float32)


# revision 19
# speedup vs baseline: 59116.6731x; 1.1544x over previous
"""ConvSpikingBlock Trainium2 kernel (8 NeuronCores, data-parallel over batch).

Per core (2 of 16 batches, 40 frames):
  phase 1 NEFF: conv via ONE fp16 matmul per frame-half (3-term split, see
    below) -> bn_stats per half -> raw stats out; host combines stats across
    cores in fp64 and folds the BN affine into the conv weights.
  phase 2 NEFF: conv with folded weights accumulates onto a PSUM-resident
    membrane; per step t, per batch b:
      ACT:  bank = beta * v_prev + b''         (per-partition bias, 1024 wide)
      PE :  bank += (-theta I) @ s_prev        (f32r reset, per 512 half)
      PE :  bank += W3.T @ rhs3[f]             (fp16 conv, per 512 half)
      DVE:  s = (bank > theta)                 (f32r {0,1}, 1024 wide)
      DMA:  s -> spk[b,t] in SBUF layout; host un-layouts to (B,T,C,H,W).

Conv precision: 3-term fp16 split. x ~ a + b with a=fp16(x), b=fp16-ish
remainder; W ~ Wa + Wb. x@W = a@Wa + (x-a)@Wa + a@(W-Wa) + O(2^-24).
The small terms are pre-scaled by 64 (operands) / 1/64 (weights) so every
fp16 operand stays in normal range (subnormal-flush-proof); products are
unchanged. Rows: rhs=[a; 64(x-a); a/64], lhsT=[Wa; Wa/64; 64(W-Wa)], K=108.

The rhs is im2col'ed on the HOST into the exact SBUF tile layout
(row k = j*6+kw*2+cin with j=r+kh, col n = g*64+w, h=4g+r), so each frame
loads with a single DMA instruction; DMA_DIRECT2D dispatch (~0.7us each on
the issuing engine queue) was the baseline bottleneck. Spikes are written
to DRAM in the SBUF layout (one DMA per (b,t)) and un-layouted on host.
"""

import glob
import json
import os
import subprocess
import sys
import tempfile
import time

sys.path.insert(0, "/opt/trn_rl_repo")

import ml_dtypes
import numpy as np

import concourse.bacc as bacc
import concourse.tile as tile
from concourse import mybir
from concourse.bass_utils import run_bass_kernel_spmd

F32 = mybir.dt.float32
F32R = mybir.dt.float32r
F16 = mybir.dt.float16
BF16 = mybir.dt.bfloat16

B, T, CIN, H, W = 16, 20, 2, 64, 64
COUT, KS = 32, 3
NC_ = 8
BLOC = B // NC_          # 2 batches per core
NF = BLOC * T            # 40 frames per core
EPS = 1e-5
K1 = 36                  # base im2col rows (6 row6 x 3 kw x 2 cin)
K3 = 108                 # 3-term rows
NPIX = 1024              # free size per frame (16 groups x 64 cols)

LAST_EXEC_NS = {}
LAST_INFO = {}
TRACE = bool(os.environ.get("KERNEL_TRACE"))


def _trunc12(a):
    return (np.ascontiguousarray(a).view(np.uint32) & np.uint32(0xFFFFF000)).view(
        np.float32
    )


def _w_block(w):
    """[36,128] weight block: k=(row6*6+kw*2+cin), m=(cout*4+r)."""
    wb = np.zeros((K1, 128), np.float64)
    for r in range(4):
        for kh in range(KS):
            k6 = r + kh
            for kw in range(KS):
                for cin in range(CIN):
                    wb[k6 * 6 + kw * 2 + cin, r::4] = w[:, cin, kh, kw]
    return wb


def _split3_rhs(r32):
    """[..., 36, n] f32 -> [..., 108, n] fp16: [a; 64(x-a); a/64]."""
    a = r32.astype(np.float16)
    a32 = a.astype(np.float32)
    bs = ((r32 - a32) * np.float32(64.0)).astype(np.float16)
    as_ = (a32 / np.float32(64.0)).astype(np.float16)
    return np.concatenate([a, bs, as_], axis=-2)


def _split3_w(wb32):
    """[36,128] f32 -> [108,128] fp16: [Wa; Wa/64; 64(W-Wa)]."""
    wa = wb32.astype(np.float16)
    wa32 = wa.astype(np.float32)
    was = (wa32 / np.float32(64.0)).astype(np.float16)
    wbs = ((wb32 - wa32) * np.float32(64.0)).astype(np.float16)
    return np.concatenate([wa, was, wbs], axis=0)


# frames whose stats go through scalar-engine Sum/SumSq instead of DVE bn_stats
SC_FRAMES = frozenset(f for f in range(NF) if f % 4 == 3)
N_SC = len(SC_FRAMES)
N_DVE = NF - N_SC


def _phase1():
    nc = bacc.Bacc("TRN2", target_bir_lowering=False, debug=False, num_devices=NC_)
    rhs_d = nc.dram_tensor("rhs", [BLOC, T, K3, NPIX], F16, kind="ExternalInput")
    w_d = nc.dram_tensor("w1", [K3, 128], F16, kind="ExternalInput")
    st_d = nc.dram_tensor("stats", [128, N_DVE * 12], F32, kind="ExternalOutput")
    ss_d = nc.dram_tensor("ssum", [128, N_SC * 2], F32, kind="ExternalOutput")

    with tile.TileContext(nc) as tc:
        with (
            tc.tile_pool(name="res", bufs=1) as res,
            tc.tile_pool(name="rp", bufs=8) as rp,
            tc.tile_pool(name="scr", bufs=2) as scr,
            tc.tile_pool(name="psum", bufs=4, space="PSUM") as psum,
        ):
            w1 = res.tile([K3, 128], F16)
            nc.sync.dma_start(w1[:], w_d[:])
            statsbuf = res.tile([128, N_DVE * 12], F32)
            ssumbuf = res.tile([128, N_SC * 2], F32)
            dve_i = 0
            sc_i = 0
            for f in range(NF):
                b, t = divmod(f, T)
                r = rp.tile([K3, NPIX], F16, name=f"r{f}", tag="r")
                eng = nc.sync if f % 2 == 0 else nc.scalar
                if f < 2:
                    # split the first loads so the pipeline starts sooner
                    eng.dma_start(r[:, 0:512], rhs_d[b, t][:, 0:512])
                    eng.dma_start(r[:, 512:1024], rhs_d[b, t][:, 512:1024])
                else:
                    eng.dma_start(r[:], rhs_d[b, t])
                acc = psum.tile([128, NPIX], F32)
                for hf in range(2):
                    cols = slice(hf * 512, hf * 512 + 512)
                    nc.tensor.matmul(
                        acc[:, cols], w1[:], r[:, cols], start=True, stop=True
                    )
                if f in SC_FRAMES:
                    scratch = scr.tile([128, NPIX], F32, name=f"scr{f}", tag="scr")
                    nc.scalar.activation(
                        scratch[:], acc[:],
                        mybir.ActivationFunctionType.Identity,
                        accum_out=ssumbuf[:, sc_i * 2 : sc_i * 2 + 1],
                    )
                    nc.scalar.activation(
                        scratch[:], acc[:],
                        mybir.ActivationFunctionType.Square,
                        accum_out=ssumbuf[:, sc_i * 2 + 1 : sc_i * 2 + 2],
                    )
                    sc_i += 1
                else:
                    for hf in range(2):
                        nc.vector.bn_stats(
                            statsbuf[:, dve_i * 12 + hf * 6 : dve_i * 12 + hf * 6 + 6],
                            acc[:, hf * 512 : hf * 512 + 512],
                        )
                    dve_i += 1
            nc.sync.dma_start(st_d[:], statsbuf[:])
            nc.sync.dma_start(ss_d[:], ssumbuf[:])
    nc.compile()
    return nc


def _phase2():
    zi = _phase2.zero_init
    nc = bacc.Bacc("TRN2", target_bir_lowering=False, debug=False, num_devices=NC_)
    rhs_d = nc.dram_tensor("rhs", [BLOC, T, K3, NPIX], F16, kind="ExternalInput")
    w_d = nc.dram_tensor("w2", [K3, 128], F16, kind="ExternalInput")
    ni_d = nc.dram_tensor("negI", [128, 128], BF16, kind="ExternalInput")
    bi_d = nc.dram_tensor("bias", [128, 1], F32, kind="ExternalInput")
    if not zi:
        vi_d = nc.dram_tensor("vinit", [BLOC, 128, NPIX], F32, kind="ExternalInput")
        s0_d = nc.dram_tensor("sinit", [BLOC, 128, NPIX], BF16, kind="ExternalInput")
    out_d = nc.dram_tensor("spk", [BLOC, T, 128, NPIX], BF16, kind="ExternalOutput")

    BETA = _phase2.beta
    THETA = _phase2.theta

    with tile.TileContext(nc) as tc:
        with (
            tc.tile_pool(name="res", bufs=1) as res,
            tc.tile_pool(name="sp", bufs=16) as sp,
            tc.tile_pool(name="psum", bufs=1, space="PSUM") as psum,
        ):
            w2 = res.tile([K3, 128], F16)
            nc.sync.dma_start(w2[:], w_d[:])
            negI = res.tile([128, 128], BF16)
            nc.sync.dma_start(negI[:], ni_d[:])
            bias = res.tile([128, 1], F32)
            nc.sync.dma_start(bias[:], bi_d[:])
            if not zi:
                vinit = res.tile([128, BLOC * NPIX], F32)
                sinit = res.tile([128, BLOC * NPIX], BF16)
                for b in range(BLOC):
                    nc.sync.dma_start(vinit[:, b * NPIX : (b + 1) * NPIX], vi_d[b])
                    nc.sync.dma_start(sinit[:, b * NPIX : (b + 1) * NPIX], s0_d[b])

            # rhs tiles, all SBUF resident; DMAs issued in consumption order
            rtiles = {}
            for t in range(T):
                for b in range(BLOC):
                    f = b * T + t
                    r = res.tile([K3, NPIX], F16, name=f"r{f}")
                    nc.sync.dma_start(r[:], rhs_d[b, t])
                    rtiles[f] = r

            banks = [
                [psum.tile([128, NPIX], F32, name=f"bk{b}_{par}") for par in range(2)]
                for b in range(BLOC)
            ]

            s_prev = {}
            if not zi:
                for b in range(BLOC):
                    s_prev[b] = sinit[:, b * NPIX : (b + 1) * NPIX]
            for t in range(T):
                cur = [banks[b][t % 2] for b in range(BLOC)]
                for b in range(BLOC):
                    if t == 0:
                        if zi:
                            # v_init == 0: bank = 0*garbage + bias
                            nc.scalar.activation(
                                cur[b][:], cur[b][:],
                                mybir.ActivationFunctionType.Identity,
                                bias=bias[:], scale=0.0,
                            )
                        else:
                            nc.scalar.activation(
                                cur[b][:], vinit[:, b * NPIX : (b + 1) * NPIX],
                                mybir.ActivationFunctionType.Identity,
                                bias=bias[:], scale=BETA,
                            )
                    else:
                        nc.scalar.activation(
                            cur[b][:], banks[b][(t + 1) % 2][:],
                            mybir.ActivationFunctionType.Identity,
                            bias=bias[:], scale=BETA,
                        )
                if t > 0 or not zi:
                    for b in range(BLOC):
                        for hf in range(2):
                            cols = slice(hf * 512, hf * 512 + 512)
                            nc.tensor.matmul(
                                cur[b][:, cols], negI[:], s_prev[b][:, cols],
                                start=False, stop=True, skip_group_check=True,
                            )
                for b in range(BLOC):
                    f = b * T + t
                    for hf in range(2):
                        cols = slice(hf * 512, hf * 512 + 512)
                        nc.tensor.matmul(
                            cur[b][:, cols], w2[:], rtiles[f][:, cols],
                            start=False, stop=True, skip_group_check=True,
                        )
                for b in range(BLOC):
                    s = sp.tile([128, NPIX], BF16, name=f"s{t}_{b}", tag="s")
                    nc.vector.tensor_scalar(
                        out=s[:], in0=cur[b][:], scalar1=THETA, scalar2=None,
                        op0=mybir.AluOpType.is_gt,
                    )
                    nc.sync.dma_start(out_d[b, t], s[:])
                    s_prev[b] = s[:]
    nc.compile()
    return nc


def _get_hook():
    try:
        from trn_agent_boot.trn_boot import _ntff_profile_via_ctypes

        return _ntff_profile_via_ctypes("/opt/axon/libaxon_pjrt.so")
    except Exception:
        return None


def _ntff_exec_ns(d):
    """Max device exec time (ns) over all profiled cores via neuron-profile."""
    try:
        neffs = glob.glob(os.path.join(d, "*.neff"))
        ntffs = sorted(glob.glob(os.path.join(d, "*device*.ntff")))
        if not neffs or not ntffs:
            return None
        best = None
        only0 = not os.environ.get("KERNEL_TRACE_ALL")
        for ntff in ntffs:
            if only0 and "device000000" not in ntff:
                continue
            jf = ntff + ".json"
            subprocess.check_call(
                ["neuron-profile", "view", "--ignore-nc-buf-usage",
                 "-s", ntff, "-n", neffs[0], "--output-format=json",
                 f"--output-file={jf}", "--ignore-dma-trace"],
                cwd=d, stdout=subprocess.DEVNULL, stderr=subprocess.DEVNULL,
            )
            with open(jf) as fh:
                summ = json.load(fh)["summary"][0]
            ns = int(float(summ["total_time"]) * 1e9)
            if best is None or ns > best:
                best = ns
        return best
    except Exception:
        return None


def _run(nc, in_maps, label):
    t0 = time.time()
    if TRACE:
        hook = _get_hook()
        if hook is not None:
            d = tempfile.mkdtemp(prefix=f"ntff_{label}_")
            ctx = None
            try:
                ctx = hook(d, None)
                ctx.__enter__()
            except Exception:
                ctx = None
            if ctx is not None:
                try:
                    r = run_bass_kernel_spmd(nc, in_maps, core_ids=list(range(NC_)))
                finally:
                    try:
                        ctx.__exit__(None, None, None)
                    except Exception:
                        pass
                wall = (time.time() - t0) * 1e9
                ns = _ntff_exec_ns(d)
                LAST_EXEC_NS[label] = ns if ns is not None else wall
                LAST_INFO[label + "_wall_ns"] = wall
                LAST_INFO[label + "_trace_dir"] = d
                return r
    r = run_bass_kernel_spmd(nc, in_maps, core_ids=list(range(NC_)))
    LAST_EXEC_NS[label] = (time.time() - t0) * 1e9
    return r


def kernel(x, mem_init, conv_w, conv_b, bn_gamma, bn_bias, beta, threshold):
    x = np.asarray(x, np.float32)
    mem_init = np.asarray(mem_init, np.float32)
    conv_w = np.asarray(conv_w, np.float32)
    bn_gamma = np.asarray(bn_gamma, np.float32)
    bn_bias = np.asarray(bn_bias, np.float32)
    betac = float(np.clip(np.float32(beta), 0.0, 1.0))
    theta = float(np.float32(threshold))

    # ---- host prep: padded input -> im2col rhs in the SBUF tile layout
    xp = np.zeros((B, T, CIN, 66, 66), np.float32)
    xp[:, :, :, 1:65, 1:65] = x
    s_ = xp.strides
    # V[b,t,j,kw,cin,g,w] = xp[b,t,cin,4g+j,kw+w]
    v = np.lib.stride_tricks.as_strided(
        xp,
        (B, T, 6, KS, CIN, 16, 64),
        (s_[0], s_[1], s_[3], s_[4], s_[2], 4 * s_[3], s_[4]),
    )
    r32 = np.ascontiguousarray(v).reshape(B, T, K1, NPIX)
    rhs_all = np.ascontiguousarray(_split3_rhs(r32))  # [B,T,108,1024] fp16

    wb = _w_block(conv_w)  # [36,128] fp64
    w1 = _split3_w(wb.astype(np.float32))

    # ---- phase 1: per-channel stats of the conv output
    nc1 = _phase1()
    in_maps1 = [
        {"rhs": rhs_all[c * BLOC : (c + 1) * BLOC], "w1": w1} for c in range(NC_)
    ]
    r1 = _run(nc1, in_maps1, "phase1")

    # ---- host: combine stats (each 6-tuple: [cnt,mean,M2, cnt,mean,M2])
    tot_s = np.zeros(COUT, np.float64)
    tot_q = np.zeros(COUT, np.float64)
    for c in range(NC_):
        st = r1.results[c]["stats"].astype(np.float64).reshape(128, N_DVE * 2, 6)
        for half in (0, 3):
            cnt = st[:, :, half]
            mean = st[:, :, half + 1]
            m2 = st[:, :, half + 2]
            tot_s += (cnt * mean).reshape(32, 4, -1).sum(axis=(1, 2))
            tot_q += (m2 + cnt * mean * mean).reshape(32, 4, -1).sum(axis=(1, 2))
        ss = r1.results[c]["ssum"].astype(np.float64).reshape(128, N_SC, 2)
        tot_s += ss[:, :, 0].reshape(32, 4, -1).sum(axis=(1, 2))
        tot_q += ss[:, :, 1].reshape(32, 4, -1).sum(axis=(1, 2))
    n_tot = float(B * T * H * W)
    mu = tot_s / n_tot
    var = tot_q / n_tot - mu * mu
    gp = bn_gamma.astype(np.float64) / np.sqrt(var + EPS)
    # reference normalizes y=conv+cb, but cb cancels: b'' = bn_bias - gp*mu
    bpp = bn_bias.astype(np.float64) - gp * mu
    wb2 = (wb * np.repeat(gp, 4)[None, :]).astype(np.float32)
    w2 = _split3_w(wb2)

    bias128 = np.repeat(bpp, 4).astype(np.float32).reshape(128, 1)
    negI = (-theta * np.eye(128, dtype=np.float32)).astype(ml_dtypes.bfloat16)

    def to_layout(a):
        # [B, C, H, W] -> [B, p=c*4+r, n=g*64+w] with h = 4g+r
        a = a.reshape(B, COUT, 16, 4, 64)
        return np.ascontiguousarray(a.transpose(0, 1, 3, 2, 4).reshape(B, 128, NPIX))

    v0 = to_layout(mem_init.astype(np.float32))
    s0 = to_layout((mem_init > theta).astype(np.float32)).astype(ml_dtypes.bfloat16)
    zero_init = bool((v0 == 0).all() and (s0 == 0).all())

    _phase2.beta = betac
    _phase2.theta = theta
    _phase2.zero_init = zero_init
    nc2 = _phase2()
    in_maps2 = []
    for c in range(NC_):
        m = {
            "rhs": rhs_all[c * BLOC : (c + 1) * BLOC],
            "w2": w2, "negI": negI, "bias": bias128,
        }
        if not zero_init:
            m["vinit"] = v0[c * BLOC : (c + 1) * BLOC]
            m["sinit"] = s0[c * BLOC : (c + 1) * BLOC]
        in_maps2.append(m)
    r2 = _run(nc2, in_maps2, "phase2")

    out = np.concatenate([r2.results[c]["spk"] for c in range(NC_)], axis=0)
    # [B,T,128,1024] bf16 {0,1} -> [B,T,C,H,W] f32: p=c*4+r, n=g*64+w, h=4g+r
    out = (
        out.reshape(B, T, COUT, 4, 16, 64)
        .transpose(0, 1, 2, 4, 3, 5)
        .reshape(B, T, COUT, H, W)
    )
    return np.ascontiguousarray(out).astype(np.float32)
